# revision 1
# baseline (speedup 1.0000x reference)
"""Trainium2 Bass kernel for nn_Attention_5480378270188.

Single-layer attention: q/k/v linear projections (torch Linear convention),
scores = q @ k^T (no 1/sqrt(d) scale), additive -1e9 mask, softmax over keys,
out = weights @ v.

Shapes (hardcoded): B=8, N=M=2048, D_MODEL=D_K=D_V=1024, fp32 inputs.

Sharding: data-parallel over batch — core b computes batch element b.
mask / W / biases are replicated to all 8 cores. No collectives.

On-device dtype strategy: all TensorE operands fp16 (full PE rate), fp32 PSUM
accumulation, softmax entirely in fp32. bq/bk are applied on-device (fused
into the projection PSUM->SBUF copies as per-partition ACT bias). bv is
applied on the host: out = softmax(scores) @ (v + bv) == softmax(scores) @ v
+ bv exactly, because softmax rows sum to 1.
"""

import sys

for _p in ("/opt/trn_rl_repo", "/opt/pypackages"):
    if _p not in sys.path:
        sys.path.insert(0, _p)

from contextlib import ExitStack

import numpy as np

import concourse.bass as bass
import concourse.tile as tile
from concourse import bacc, mybir
from concourse.bass import ds, ts
from concourse.bass_utils import run_bass_kernel_spmd
from concourse.masks import make_identity

P = 128
B = 8
N = 2048  # queries
M = 2048  # keys
D = 1024  # d_model
DK = 1024  # key/query dim
DV = 1024  # value dim
F = 512  # matmul moving free dim
DT = mybir.dt.float16
F32 = mybir.dt.float32
I32 = mybir.dt.int32

NEG = -1.0e9

N_BLOCKS = N // P  # 16
M_BLOCKS = M // P  # 16
D_O = D // P  # 8
DK_O = DK // P  # 8
N_MEGA = N // F  # 4 query mega-blocks (512 rows)
M_GRP = M // F  # 4 key groups (512 rows)
SC_CHUNKS = M // F  # 4 score chunks per row-block
PV_CHUNKS = DV // F  # 2


def _transpose_rowtile(nc, psA, ident32, src_f32, dst_f16, r):
    """Transpose a [128, D] fp32 row-tile into dst_f16[:, :, r*128:(r+1)*128].

    dst_f16 layout: [128(d_inner), D//128(d_outer), cols] fp16.
    Two [128, 4, 128] PSUM tiles (1 bank each), cast fp32->fp16 on the copy.
    """
    for half in range(2):
        pst = psA.tile([P, 4, P], F32, tag="ps_big")
        for j in range(4):
            db = half * 4 + j
            nc.tensor.transpose(pst[:, j, :], src_f32[:, ts(db, P)], ident32[:])
        nc.any.tensor_copy(dst_f16[:, half * 4 : half * 4 + 4, ts(r, P)], pst[:])


def build():
    nc = bacc.Bacc("TRN2", target_bir_lowering=False, debug=False)

    querys_e = nc.dram_tensor("querys", [N, D], F32, kind="ExternalInput").ap()
    keys_e = nc.dram_tensor("keys", [M, D], F32, kind="ExternalInput").ap()
    values_e = nc.dram_tensor("values", [M, D], F32, kind="ExternalInput").ap()
    mask_e = nc.dram_tensor("mask", [N, M], I32, kind="ExternalInput").ap()
    Wq_e = nc.dram_tensor("Wq", [DK, D], F32, kind="ExternalInput").ap()
    Wk_e = nc.dram_tensor("Wk", [DK, D], F32, kind="ExternalInput").ap()
    Wv_e = nc.dram_tensor("Wv", [DV, D], F32, kind="ExternalInput").ap()
    bq_e = nc.dram_tensor("bq", [DK], F32, kind="ExternalInput").ap()
    bk_e = nc.dram_tensor("bk", [DK], F32, kind="ExternalInput").ap()
    out_e = nc.dram_tensor("out", [N, DV], F32, kind="ExternalOutput").ap()

    with tile.TileContext(nc) as tc, ExitStack() as ctx:
        const = ctx.enter_context(tc.tile_pool(name="const", bufs=1))
        persist = ctx.enter_context(tc.tile_pool(name="persist", bufs=1))
        psA = ctx.enter_context(tc.tile_pool(name="psA", bufs=2, space="PSUM"))
        psSC = ctx.enter_context(tc.tile_pool(name="psSC", bufs=4, space="PSUM"))
        psPV = ctx.enter_context(tc.tile_pool(name="psPV", bufs=1, space="PSUM"))

        ident32 = const.tile([P, P], F32, tag="id32")
        make_identity(nc, ident32[:])
        ident16 = const.tile([P, P], DT, tag="id16")
        make_identity(nc, ident16[:])

        bq_sb = const.tile([P, DK_O], F32, tag="bq")
        nc.sync.dma_start(bq_sb[:], bq_e.rearrange("(o p) -> p o", p=P))
        bk_sb = const.tile([P, DK_O], F32, tag="bk")
        nc.sync.dma_start(bk_sb[:], bk_e.rearrange("(o p) -> p o", p=P))

        # persistent fp16 operands for the attention matmuls
        kT_sb = persist.tile([P, DK_O, M], DT, tag="kT")  # [dk_i, dk_o, m]
        v_sb = persist.tile([P, M_BLOCKS, DV], DT, tag="v")  # [m_i, m_o, dv]
        WqT_sb = persist.tile([P, D_O, DK], DT, tag="WqT")  # [d_i, d_o, dk]

        # ---------------- Phase A: weights + K/V projections ----------------
        with tc.tile_pool(name="phA", bufs=2) as pa:

            def load_weight_T(w_ext, dst):
                for dko in range(DK_O):
                    wst = pa.tile([P, D], F32, tag="wstage")
                    nc.sync.dma_start(wst[:], w_ext[ts(dko, P), :])
                    _transpose_rowtile(nc, psA, ident32, wst, dst, dko)

            WkT_sb = pa.tile([P, D_O, DK], DT, tag="WkT")
            WvT_sb = pa.tile([P, D_O, DV], DT, tag="WvT")
            load_weight_T(Wq_e, WqT_sb)
            load_weight_T(Wk_e, WkT_sb)
            load_weight_T(Wv_e, WvT_sb)

            for grp in range(M_GRP):
                ktT = pa.tile([P, D_O, F], DT, tag="ktT")  # keys^T staging
                vtT = pa.tile([P, D_O, F], DT, tag="vtT")  # values^T staging
                for r in range(4):
                    m0 = grp * F + r * P
                    kst = pa.tile([P, D], F32, tag="rowstage")
                    nc.sync.dma_start(kst[:], keys_e[ds(m0, P), :])
                    _transpose_rowtile(nc, psA, ident32, kst, ktT, r)
                    vst = pa.tile([P, D], F32, tag="rowstage")
                    nc.sync.dma_start(vst[:], values_e[ds(m0, P), :])
                    _transpose_rowtile(nc, psA, ident32, vst, vtT, r)

                # k projection: kT[dk, m-group] += bk
                for dko in range(DK_O):
                    ps = psSC.tile([P, F], F32, tag="ps_sc")
                    for do in range(D_O):
                        nc.tensor.matmul(
                            ps[:],
                            WkT_sb[:, do, ts(dko, P)],
                            ktT[:, do, :],
                            start=(do == 0),
                            stop=(do == D_O - 1),
                        )
                    nc.scalar.add(
                        kT_sb[:, dko, ds(grp * F, F)], ps[:], bk_sb[:, dko : dko + 1]
                    )

                # v projection: v[m, dv] (bv applied on host)
                for r in range(4):
                    mo = grp * 4 + r
                    for c in range(PV_CHUNKS):
                        ps = psSC.tile([P, F], F32, tag="ps_sc")
                        for do in range(D_O):
                            nc.tensor.matmul(
                                ps[:],
                                vtT[:, do, ts(r, P)],
                                WvT_sb[:, do, ts(c, F)],
                                start=(do == 0),
                                stop=(do == D_O - 1),
                            )
                        nc.any.tensor_copy(v_sb[:, mo, ts(c, F)], ps[:])

        # ---------------- Phase B: Q projection + attention ----------------
        with tc.tile_pool(name="mainp", bufs=2) as mp:
            for g in range(N_MEGA):
                # q projection for rows [g*512, (g+1)*512) -> qT [dk, n-chunk]
                qTt = mp.tile([P, D_O, F], DT, tag="querysT")
                for r in range(4):
                    qst = mp.tile([P, D], F32, tag="qstage")
                    nc.sync.dma_start(qst[:], querys_e[ds(g * F + r * P, P), :])
                    _transpose_rowtile(nc, psA, ident32, qst, qTt, r)
                qT = mp.tile([P, DK_O, F], DT, tag="qT")
                for dko in range(DK_O):
                    ps = psSC.tile([P, F], F32, tag="ps_sc")
                    for do in range(D_O):
                        nc.tensor.matmul(
                            ps[:],
                            WqT_sb[:, do, ts(dko, P)],
                            qTt[:, do, :],
                            start=(do == 0),
                            stop=(do == D_O - 1),
                        )
                    nc.scalar.add(qT[:, dko, :], ps[:], bq_sb[:, dko : dko + 1])

                for s in range(4):
                    blk = g * 4 + s
                    # additive mask bias: (mask - 1) * 1e9  ->  {0, -1e9}
                    mtile = mp.tile([P, M], I32, tag="mask")
                    nc.sync.dma_start(mtile[:], mask_e[ds(blk * P, P), :])
                    btile = mp.tile([P, M], F32, tag="maskbias")
                    nc.scalar.activation(
                        btile[:],
                        mtile[:],
                        mybir.ActivationFunctionType.Copy,
                        bias=NEG,
                        scale=-NEG,
                    )

                    stats = mp.tile([P, SC_CHUNKS], F32, tag="stats")
                    sums = mp.tile([P, SC_CHUNKS], F32, tag="sums")
                    negmax = mp.tile([P, 1], F32, tag="negmax")
                    rsum = mp.tile([P, 1], F32, tag="rsum")
                    rinv = mp.tile([P, 1], F32, tag="rinv")
                    w16 = mp.tile([P, M], DT, tag="w16")

                    sc_tiles = []
                    for mc in range(SC_CHUNKS):
                        ps = psSC.tile([P, F], F32, tag="ps_sc")
                        sc_tiles.append(ps)
                        for dko in range(DK_O):
                            nc.tensor.matmul(
                                ps[:],
                                qT[:, dko, ts(s, P)],
                                kT_sb[:, dko, ts(mc, F)],
                                start=(dko == 0),
                                stop=(dko == DK_O - 1),
                            )
                        nc.vector.tensor_add(ps[:], ps[:], btile[:, ts(mc, F)])
                        nc.vector.reduce_max(
                            stats[:, mc : mc + 1], ps[:], axis=mybir.AxisListType.X
                        )
                    nc.vector.reduce_max(
                        negmax[:], stats[:], axis=mybir.AxisListType.X, negate=True
                    )
                    for mc in range(SC_CHUNKS):
                        nc.scalar.activation(
                            w16[:, ts(mc, F)],
                            sc_tiles[mc][:],
                            mybir.ActivationFunctionType.Exp,
                            bias=negmax[:, 0:1],
                            scale=1.0,
                            accum_out=sums[:, mc : mc + 1],
                        )
                    nc.vector.reduce_sum(
                        rsum[:], sums[:], axis=mybir.AxisListType.X
                    )
                    nc.vector.reciprocal(rinv[:], rsum[:])

                    # transpose the probability tiles: wT[m, n-block]
                    wT = mp.tile([P, M_BLOCKS, P], DT, tag="wT")
                    for q4 in range(4):
                        pw = psA.tile([P, 4, P], DT, tag="ps_big")
                        for j in range(4):
                            mo = q4 * 4 + j
                            nc.tensor.transpose(
                                pw[:, j, :], w16[:, ts(mo, P)], ident16[:]
                            )
                        nc.vector.tensor_copy(wT[:, q4 * 4 : q4 * 4 + 4, :], pw[:])

                    # PV: out[n-block, dv] = wT.T @ v
                    pv = psPV.tile([P, PV_CHUNKS, F], F32, tag="ps_pv")
                    for c in range(PV_CHUNKS):
                        for mo in range(M_BLOCKS):
                            nc.tensor.matmul(
                                pv[:, c, :],
                                wT[:, mo, :],
                                v_sb[:, mo, ts(c, F)],
                                start=(mo == 0),
                                stop=(mo == M_BLOCKS - 1),
                            )
                    outt = mp.tile([P, DV], F32, tag="outt")
                    for c in range(PV_CHUNKS):
                        nc.vector.tensor_scalar_mul(
                            outt[:, ts(c, F)], pv[:, c, :], rinv[:, 0:1]
                        )
                    nc.sync.dma_start(out_e[ds(blk * P, P), :], outt[:])

    nc.compile()
    return nc


_CACHE = {}


def _get_nc():
    if "nc" not in _CACHE:
        _CACHE["nc"] = build()
    return _CACHE["nc"]


def run(inputs, trace=False, trace_kwargs=None):
    nc = _get_nc()
    querys = np.ascontiguousarray(np.asarray(inputs["querys"], dtype=np.float32))
    keys = np.ascontiguousarray(np.asarray(inputs["keys"], dtype=np.float32))
    values = np.ascontiguousarray(np.asarray(inputs["values"], dtype=np.float32))
    mask = np.ascontiguousarray(np.asarray(inputs["mask"], dtype=np.int32))
    shared = {
        "mask": mask,
        "Wq": np.asarray(inputs["Wq"], dtype=np.float32),
        "Wk": np.asarray(inputs["Wk"], dtype=np.float32),
        "Wv": np.asarray(inputs["Wv"], dtype=np.float32),
        "bq": np.asarray(inputs["bq"], dtype=np.float32),
        "bk": np.asarray(inputs["bk"], dtype=np.float32),
    }
    in_maps = [
        {
            "querys": querys[b],
            "keys": keys[b],
            "values": values[b],
            **shared,
        }
        for b in range(B)
    ]
    res = run_bass_kernel_spmd(
        nc,
        in_maps,
        list(range(B)),
        trace=trace,
        **(trace_kwargs or {}),
    )
    out = np.stack([res.results[b]["out"] for b in range(B)]).astype(np.float32)
    # bv folded in on the host: softmax rows sum to 1, so W @ (v + bv) = W @ v + bv
    out += np.asarray(inputs["bv"], dtype=np.float32)[None, None, :]
    return out, res


def kernel(**inputs) -> np.ndarray:
    out, _ = run(inputs, trace=False)
    return out


if __name__ == "__main__":
    nc = _get_nc()
    print("built + compiled OK")


# revision 2
# speedup vs baseline: 1.0244x; 1.0244x over previous
"""Trainium2 Bass kernel for nn_Attention_5480378270188.

Single-layer attention: q/k/v linear projections (torch Linear convention),
scores = q @ k^T (no 1/sqrt(d) scale), additive -1e9 mask, softmax over keys,
out = weights @ v.

Shapes (hardcoded): B=8, N=M=2048, D_MODEL=D_K=D_V=1024, fp32 inputs.

Sharding: data-parallel over batch — core b computes batch element b.
mask / W / biases are replicated to all 8 cores. No collectives.

On-device dtype strategy: all TensorE operands fp16 (full PE rate), fp32 PSUM
accumulation, softmax entirely in fp32. All operand transposes run on the DMA
X-bar (dma transpose, 16-bit) after an in-flight fp32->fp16 SWDGE cast-DMA
through a DRAM bounce — the PE does nothing but matmuls. bq/bk are applied
on-device (fused into the projection PSUM->SBUF copies as per-partition ACT
bias). bv is applied on the host: softmax rows sum to 1, so
softmax(s) @ (v + bv) == softmax(s) @ v + bv exactly.
"""

import sys

for _p in ("/opt/trn_rl_repo", "/opt/pypackages"):
    if _p not in sys.path:
        sys.path.insert(0, _p)

from contextlib import ExitStack

import numpy as np

import concourse.bass as bass
import concourse.tile as tile
from concourse import bacc, mybir
from concourse.bass import ds, ts
from concourse.bass_utils import run_bass_kernel_spmd

P = 128
B = 8
N = 2048  # queries
M = 2048  # keys
D = 1024  # d_model
DK = 1024  # key/query dim
DV = 1024  # value dim
F = 512  # matmul moving free dim
DT = mybir.dt.float16
F32 = mybir.dt.float32
I32 = mybir.dt.int32

NEG = -1.0e9

N_BLOCKS = N // P  # 16
M_BLOCKS = M // P  # 16
D_O = D // P  # 8
DK_O = DK // P  # 8
N_MEGA = N // F  # 4 query mega-blocks (512 rows)
M_GRP = M // F  # 4 key groups (512 rows)
SC_CHUNKS = M // F  # 4 score chunks per row-block
PV_CHUNKS = DV // F  # 2


def build():
    nc = bacc.Bacc("TRN2", target_bir_lowering=False, debug=False)

    querys_e = nc.dram_tensor("querys", [N, D], F32, kind="ExternalInput").ap()
    keys_e = nc.dram_tensor("keys", [M, D], F32, kind="ExternalInput").ap()
    values_e = nc.dram_tensor("values", [M, D], F32, kind="ExternalInput").ap()
    mask_e = nc.dram_tensor("mask", [N, M], I32, kind="ExternalInput").ap()
    Wq_e = nc.dram_tensor("Wq", [DK, D], F32, kind="ExternalInput").ap()
    Wk_e = nc.dram_tensor("Wk", [DK, D], F32, kind="ExternalInput").ap()
    Wv_e = nc.dram_tensor("Wv", [DV, D], F32, kind="ExternalInput").ap()
    bq_e = nc.dram_tensor("bq", [DK], F32, kind="ExternalInput").ap()
    bk_e = nc.dram_tensor("bk", [DK], F32, kind="ExternalInput").ap()
    out_e = nc.dram_tensor("out", [N, DV], F32, kind="ExternalOutput").ap()

    with tile.TileContext(nc) as tc, ExitStack() as ctx:
        const = ctx.enter_context(tc.tile_pool(name="const", bufs=1))
        persist = ctx.enter_context(tc.tile_pool(name="persist", bufs=1))
        dram = ctx.enter_context(tc.tile_pool(name="dram", bufs=1, space="DRAM"))
        psSC = ctx.enter_context(tc.tile_pool(name="psSC", bufs=4, space="PSUM"))
        psPV = ctx.enter_context(tc.tile_pool(name="psPV", bufs=2, space="PSUM"))

        bq_sb = const.tile([P, DK_O], F32, tag="bq")
        nc.sync.dma_start(bq_sb[:], bq_e.rearrange("(o p) -> p o", p=P))
        bk_sb = const.tile([P, DK_O], F32, tag="bk")
        nc.sync.dma_start(bk_sb[:], bk_e.rearrange("(o p) -> p o", p=P))

        # persistent fp16 operands for the attention matmuls
        kT_sb = persist.tile([P, DK_O, M], DT, tag="kT")  # [dk_i, dk_o, m]
        v_sb = persist.tile([P, M_BLOCKS, DV], DT, tag="v")  # [m_i, m_o, dv]
        WqT_sb = persist.tile([P, D_O, DK], DT, tag="WqT")  # [d_i, d_o, dk]

        # fp16 DRAM bounce copies (SWDGE cast fp32 -> fp16 in flight)
        q16_d = dram.tile([N, D], DT, tag="q16")
        k16_d = dram.tile([M, D], DT, tag="k16")
        v16_d = dram.tile([M, D], DT, tag="v16")
        nc.gpsimd.dma_start(k16_d[:], keys_e[:])
        nc.gpsimd.dma_start(v16_d[:], values_e[:])
        nc.gpsimd.dma_start(q16_d[:], querys_e[:])

        # ---------------- Phase A: weights + K/V projections ----------------
        with tc.tile_pool(name="phA", bufs=2) as pa:

            def load_weight_T(w_ext, dst, tag):
                w16 = dram.tile([DK, D], DT, tag=tag)
                nc.gpsimd.dma_start(w16[:], w_ext[:])
                # one X-bar transpose: [dk, d] -> [d_i, d_o, dk]
                nc.sync.dma_start(dst[:], w16[:], transpose=True)

            WkT_sb = pa.tile([P, D_O, DK], DT, tag="WkT")
            WvT_sb = pa.tile([P, D_O, DV], DT, tag="WvT")
            load_weight_T(Wq_e, WqT_sb, "wq16")
            load_weight_T(Wk_e, WkT_sb, "wk16")
            load_weight_T(Wv_e, WvT_sb, "wv16")

            for grp in range(M_GRP):
                # X-bar transpose 512 rows of keys/values: -> [d_i, d_o, m(512)]
                ktT = pa.tile([P, D_O, F], DT, tag="ktT")
                nc.sync.dma_start(
                    ktT[:], k16_d[ds(grp * F, F), :], transpose=True
                )
                vtT = pa.tile([P, D_O, F], DT, tag="vtT")
                nc.sync.dma_start(
                    vtT[:], v16_d[ds(grp * F, F), :], transpose=True
                )

                # k projection: kT[dk, m-group] += bk
                for dko in range(DK_O):
                    ps = psSC.tile([P, F], F32, tag="ps_sc")
                    for do in range(D_O):
                        nc.tensor.matmul(
                            ps[:],
                            WkT_sb[:, do, ts(dko, P)],
                            ktT[:, do, :],
                            start=(do == 0),
                            stop=(do == D_O - 1),
                        )
                    nc.scalar.add(
                        kT_sb[:, dko, ds(grp * F, F)], ps[:], bk_sb[:, dko : dko + 1]
                    )

                # v projection: v[m, dv] (bv applied on host)
                for r in range(4):
                    mo = grp * 4 + r
                    for c in range(PV_CHUNKS):
                        ps = psSC.tile([P, F], F32, tag="ps_sc")
                        for do in range(D_O):
                            nc.tensor.matmul(
                                ps[:],
                                vtT[:, do, ts(r, P)],
                                WvT_sb[:, do, ts(c, F)],
                                start=(do == 0),
                                stop=(do == D_O - 1),
                            )
                        nc.any.tensor_copy(v_sb[:, mo, ts(c, F)], ps[:])

        # ---------------- Phase B: Q projection + attention ----------------
        with tc.tile_pool(name="mainp", bufs=2) as mp:
            for g in range(N_MEGA):
                # q projection for rows [g*512, (g+1)*512) -> qT [dk, n-chunk]
                qTt = mp.tile([P, D_O, F], DT, tag="querysT")
                nc.sync.dma_start(
                    qTt[:], q16_d[ds(g * F, F), :], transpose=True
                )
                qT = mp.tile([P, DK_O, F], DT, tag="qT")
                for dko in range(DK_O):
                    ps = psSC.tile([P, F], F32, tag="ps_sc")
                    for do in range(D_O):
                        nc.tensor.matmul(
                            ps[:],
                            WqT_sb[:, do, ts(dko, P)],
                            qTt[:, do, :],
                            start=(do == 0),
                            stop=(do == D_O - 1),
                        )
                    nc.scalar.add(qT[:, dko, :], ps[:], bq_sb[:, dko : dko + 1])

                for s in range(4):
                    blk = g * 4 + s
                    # additive mask bias: (mask - 1) * 1e9  ->  {0, -1e9}
                    mtile = mp.tile([P, M], I32, tag="mask")
                    nc.sync.dma_start(mtile[:], mask_e[ds(blk * P, P), :])
                    btile = mp.tile([P, M], F32, tag="maskbias")
                    nc.scalar.activation(
                        btile[:],
                        mtile[:],
                        mybir.ActivationFunctionType.Copy,
                        bias=NEG,
                        scale=-NEG,
                    )

                    stats = mp.tile([P, SC_CHUNKS], F32, tag="stats")
                    sums = mp.tile([P, SC_CHUNKS], F32, tag="sums")
                    negmax = mp.tile([P, 1], F32, tag="negmax")
                    rsum = mp.tile([P, 1], F32, tag="rsum")
                    rinv = mp.tile([P, 1], F32, tag="rinv")
                    w16 = mp.tile([P, M], DT, tag="w16")

                    sc_tiles = []
                    for mc in range(SC_CHUNKS):
                        ps = psSC.tile([P, F], F32, tag="ps_sc")
                        sc_tiles.append(ps)
                        for dko in range(DK_O):
                            nc.tensor.matmul(
                                ps[:],
                                qT[:, dko, ts(s, P)],
                                kT_sb[:, dko, ts(mc, F)],
                                start=(dko == 0),
                                stop=(dko == DK_O - 1),
                            )
                        nc.vector.tensor_add(ps[:], ps[:], btile[:, ts(mc, F)])
                        nc.vector.reduce_max(
                            stats[:, mc : mc + 1], ps[:], axis=mybir.AxisListType.X
                        )
                    nc.vector.reduce_max(
                        negmax[:], stats[:], axis=mybir.AxisListType.X, negate=True
                    )
                    for mc in range(SC_CHUNKS):
                        nc.scalar.activation(
                            w16[:, ts(mc, F)],
                            sc_tiles[mc][:],
                            mybir.ActivationFunctionType.Exp,
                            bias=negmax[:, 0:1],
                            scale=1.0,
                            accum_out=sums[:, mc : mc + 1],
                        )
                    nc.vector.reduce_sum(
                        rsum[:], sums[:], axis=mybir.AxisListType.X
                    )
                    nc.vector.reciprocal(rinv[:], rsum[:])

                    # X-bar transpose of the probability tiles: [n, m] -> [m_i, m_o, n]
                    wT = mp.tile([P, M_BLOCKS, P], DT, tag="wT")
                    nc.sync.dma_start(wT[:], w16[:], transpose=True)

                    # PV: out[n-block, dv] = wT.T @ v
                    pv = psPV.tile([P, PV_CHUNKS, F], F32, tag="ps_pv")
                    for c in range(PV_CHUNKS):
                        for mo in range(M_BLOCKS):
                            nc.tensor.matmul(
                                pv[:, c, :],
                                wT[:, mo, :],
                                v_sb[:, mo, ts(c, F)],
                                start=(mo == 0),
                                stop=(mo == M_BLOCKS - 1),
                            )
                    outt = mp.tile([P, DV], F32, tag="outt")
                    for c in range(PV_CHUNKS):
                        nc.vector.tensor_scalar_mul(
                            outt[:, ts(c, F)], pv[:, c, :], rinv[:, 0:1]
                        )
                    nc.sync.dma_start(out_e[ds(blk * P, P), :], outt[:])

    nc.compile()
    return nc


_CACHE = {}


def _get_nc():
    if "nc" not in _CACHE:
        _CACHE["nc"] = build()
    return _CACHE["nc"]


def run(inputs, trace=False, trace_kwargs=None):
    nc = _get_nc()
    querys = np.ascontiguousarray(np.asarray(inputs["querys"], dtype=np.float32))
    keys = np.ascontiguousarray(np.asarray(inputs["keys"], dtype=np.float32))
    values = np.ascontiguousarray(np.asarray(inputs["values"], dtype=np.float32))
    mask = np.ascontiguousarray(np.asarray(inputs["mask"], dtype=np.int32))
    shared = {
        "mask": mask,
        "Wq": np.asarray(inputs["Wq"], dtype=np.float32),
        "Wk": np.asarray(inputs["Wk"], dtype=np.float32),
        "Wv": np.asarray(inputs["Wv"], dtype=np.float32),
        "bq": np.asarray(inputs["bq"], dtype=np.float32),
        "bk": np.asarray(inputs["bk"], dtype=np.float32),
    }
    in_maps = [
        {
            "querys": querys[b],
            "keys": keys[b],
            "values": values[b],
            **shared,
        }
        for b in range(B)
    ]
    res = run_bass_kernel_spmd(
        nc,
        in_maps,
        list(range(B)),
        trace=trace,
        **(trace_kwargs or {}),
    )
    out = np.stack([res.results[b]["out"] for b in range(B)]).astype(np.float32)
    # bv folded in on the host: softmax rows sum to 1, so W @ (v + bv) = W @ v + bv
    out += np.asarray(inputs["bv"], dtype=np.float32)[None, None, :]
    return out, res


def kernel(**inputs) -> np.ndarray:
    out, _ = run(inputs, trace=False)
    return out


if __name__ == "__main__":
    nc = _get_nc()
    print("built + compiled OK")


# revision 4
# speedup vs baseline: 1.1176x; 1.0911x over previous
"""Trainium2 Bass kernel for nn_Attention_5480378270188.

Single-layer attention: q/k/v linear projections (torch Linear convention),
scores = q @ k^T (no 1/sqrt(d) scale), additive -1e9 mask, softmax over keys,
out = weights @ v.

Shapes (hardcoded): B=8, N=M=2048, D_MODEL=D_K=D_V=1024, fp32 inputs.

Sharding: data-parallel over batch — core b computes batch element b.
mask / W / biases are replicated to all 8 cores. No collectives.

On-device dtype strategy: all TensorE operands fp16 (full PE rate), fp32 PSUM
accumulation, softmax entirely in fp32. All operand transposes run on the DMA
X-bar (dma transpose, 16-bit) after an in-flight fp32->fp16 SWDGE cast-DMA
through a DRAM bounce — the PE does nothing but matmuls. bq/bk are applied
on-device (fused into the projection PSUM->SBUF copies as per-partition ACT
bias). bv is applied on the host: softmax rows sum to 1, so
softmax(s) @ (v + bv) == softmax(s) @ v + bv exactly.
"""

import sys

for _p in ("/opt/trn_rl_repo", "/opt/pypackages"):
    if _p not in sys.path:
        sys.path.insert(0, _p)

from contextlib import ExitStack

import numpy as np

import concourse.bass as bass
import concourse.tile as tile
from concourse import bacc, mybir
from concourse.bass import ds, ts
from concourse.bass_utils import run_bass_kernel_spmd

P = 128
B = 8
N = 2048  # queries
M = 2048  # keys
D = 1024  # d_model
DK = 1024  # key/query dim
DV = 1024  # value dim
F = 512  # matmul moving free dim
DT = mybir.dt.float16
F32 = mybir.dt.float32
I32 = mybir.dt.int32

NEG = -1.0e9

N_BLOCKS = N // P  # 16
M_BLOCKS = M // P  # 16
D_O = D // P  # 8
DK_O = DK // P  # 8
N_MEGA = N // F  # 4 query mega-blocks (512 rows)
M_GRP = M // F  # 4 key groups (512 rows)
SC_CHUNKS = M // F  # 4 score chunks per row-block
PV_CHUNKS = DV // F  # 2


def build():
    nc = bacc.Bacc("TRN2", target_bir_lowering=False, debug=False)

    querys_e = nc.dram_tensor("querys", [N, D], F32, kind="ExternalInput").ap()
    keys_e = nc.dram_tensor("keys", [M, D], F32, kind="ExternalInput").ap()
    values_e = nc.dram_tensor("values", [M, D], F32, kind="ExternalInput").ap()
    mask_e = nc.dram_tensor("mask", [N, M], I32, kind="ExternalInput").ap()
    Wq_e = nc.dram_tensor("Wq", [DK, D], F32, kind="ExternalInput").ap()
    Wk_e = nc.dram_tensor("Wk", [DK, D], F32, kind="ExternalInput").ap()
    Wv_e = nc.dram_tensor("Wv", [DV, D], F32, kind="ExternalInput").ap()
    bq_e = nc.dram_tensor("bq", [DK], F32, kind="ExternalInput").ap()
    bk_e = nc.dram_tensor("bk", [DK], F32, kind="ExternalInput").ap()
    out_e = nc.dram_tensor("out", [N, DV], F32, kind="ExternalOutput").ap()

    with tile.TileContext(nc) as tc, ExitStack() as ctx:
        const = ctx.enter_context(tc.tile_pool(name="const", bufs=1))
        persist = ctx.enter_context(tc.tile_pool(name="persist", bufs=1))
        dram = ctx.enter_context(tc.tile_pool(name="dram", bufs=1, space="DRAM"))
        psSC = ctx.enter_context(tc.tile_pool(name="psSC", bufs=6, space="PSUM"))
        psPV = ctx.enter_context(tc.tile_pool(name="psPV", bufs=1, space="PSUM"))

        bq_sb = const.tile([P, DK_O], F32, tag="bq")
        nc.sync.dma_start(bq_sb[:], bq_e.rearrange("(o p) -> p o", p=P))
        bk_sb = const.tile([P, DK_O], F32, tag="bk")
        nc.sync.dma_start(bk_sb[:], bk_e.rearrange("(o p) -> p o", p=P))

        # persistent fp16 operands for the attention matmuls
        kT_sb = persist.tile([P, DK_O, M], DT, tag="kT")  # [dk_i, dk_o, m]
        v_sb = persist.tile([P, M_BLOCKS, DV], DT, tag="v")  # [m_i, m_o, dv]
        qT_sb = persist.tile([P, DK_O, N], DT, tag="qT")  # [dk_i, dk_o, n]

        # fp16 DRAM bounces (SWDGE casts fp32 -> fp16 in flight). Weights
        # first — they gate every projection.
        wq16 = dram.tile([DK, D], DT, tag="wq16")
        nc.gpsimd.dma_start(wq16[:], Wq_e[:])
        wk16 = dram.tile([DK, D], DT, tag="wk16")
        nc.gpsimd.dma_start(wk16[:], Wk_e[:])
        wv16 = dram.tile([DV, D], DT, tag="wv16")
        nc.gpsimd.dma_start(wv16[:], Wv_e[:])
        q16_d = dram.tile([N, D], DT, tag="q16")
        k16_d = dram.tile([M, D], DT, tag="k16")
        v16_d = dram.tile([M, D], DT, tag="v16")
        for g in range(N_MEGA):
            nc.gpsimd.dma_start(q16_d[ds(g * F, F), :], querys_e[ds(g * F, F), :])
        for g in range(M_GRP):
            nc.gpsimd.dma_start(k16_d[ds(g * F, F), :], keys_e[ds(g * F, F), :])
            nc.gpsimd.dma_start(v16_d[ds(g * F, F), :], values_e[ds(g * F, F), :])

        # ---------------- Phase A: all projections ----------------
        with (
            tc.tile_pool(name="phW", bufs=1) as pw,
            tc.tile_pool(name="phA", bufs=2) as pa,
        ):
            WqT_sb = pw.tile([P, D_O, DK], DT, tag="WqT")
            nc.sync.dma_start(WqT_sb[:], wq16[:], transpose=True)
            WkT_sb = pw.tile([P, D_O, DK], DT, tag="WkT")
            nc.sync.dma_start(WkT_sb[:], wk16[:], transpose=True)
            WvT_sb = pw.tile([P, D_O, DV], DT, tag="WvT")
            nc.sync.dma_start(WvT_sb[:], wv16[:], transpose=True)

            # q projections (only need WqT + q16) — fills the PE while k/v
            # casts are still in flight
            for g in range(N_MEGA):
                qTt = pa.tile([P, D_O, F], DT, tag="querysT")
                nc.sync.dma_start(qTt[:], q16_d[ds(g * F, F), :], transpose=True)
                for dko in range(DK_O):
                    ps = psSC.tile([P, F], F32, tag="ps_sc")
                    for do in range(D_O):
                        nc.tensor.matmul(
                            ps[:],
                            WqT_sb[:, do, ts(dko, P)],
                            qTt[:, do, :],
                            start=(do == 0),
                            stop=(do == D_O - 1),
                        )
                    nc.scalar.add(
                        qT_sb[:, dko, ds(g * F, F)], ps[:], bq_sb[:, dko : dko + 1]
                    )

            for grp in range(M_GRP):
                ktT = pa.tile([P, D_O, F], DT, tag="ktT")
                nc.sync.dma_start(ktT[:], k16_d[ds(grp * F, F), :], transpose=True)
                vtT = pa.tile([P, D_O, F], DT, tag="vtT")
                nc.sync.dma_start(vtT[:], v16_d[ds(grp * F, F), :], transpose=True)

                # k projection: kT[dk, m-group] += bk
                for dko in range(DK_O):
                    ps = psSC.tile([P, F], F32, tag="ps_sc")
                    for do in range(D_O):
                        nc.tensor.matmul(
                            ps[:],
                            WkT_sb[:, do, ts(dko, P)],
                            ktT[:, do, :],
                            start=(do == 0),
                            stop=(do == D_O - 1),
                        )
                    nc.scalar.add(
                        kT_sb[:, dko, ds(grp * F, F)], ps[:], bk_sb[:, dko : dko + 1]
                    )

                # v projection: v[m, dv] (bv applied on host)
                for r in range(4):
                    mo = grp * 4 + r
                    for c in range(PV_CHUNKS):
                        ps = psSC.tile([P, F], F32, tag="ps_sc")
                        for do in range(D_O):
                            nc.tensor.matmul(
                                ps[:],
                                vtT[:, do, ts(r, P)],
                                WvT_sb[:, do, ts(c, F)],
                                start=(do == 0),
                                stop=(do == D_O - 1),
                            )
                        nc.any.tensor_copy(v_sb[:, mo, ts(c, F)], ps[:])

        # ---------------- Phase B: attention blocks ----------------
        with tc.tile_pool(name="mainp", bufs=2) as mp:
            for blk in range(N_BLOCKS):
                s = blk % 4
                # additive mask bias: (mask - 1) * 1e9  ->  {0, -1e9}
                mtile = mp.tile([P, M], I32, tag="mask")
                nc.sync.dma_start(mtile[:], mask_e[ds(blk * P, P), :])
                btile = mp.tile([P, M], F32, tag="maskbias")
                nc.scalar.activation(
                    btile[:],
                    mtile[:],
                    mybir.ActivationFunctionType.Copy,
                    bias=NEG,
                    scale=-NEG,
                )

                stats = mp.tile([P, SC_CHUNKS], F32, tag="stats")
                sums = mp.tile([P, SC_CHUNKS], F32, tag="sums")
                negmax = mp.tile([P, 1], F32, tag="negmax")
                rsum = mp.tile([P, 1], F32, tag="rsum")
                rinv = mp.tile([P, 1], F32, tag="rinv")
                w16 = mp.tile([P, M], DT, tag="w16")

                sc_tiles = []
                for mc in range(SC_CHUNKS):
                    ps = psSC.tile([P, F], F32, tag="ps_sc")
                    sc_tiles.append(ps)
                    for dko in range(DK_O):
                        nc.tensor.matmul(
                            ps[:],
                            qT_sb[:, dko, ds(blk * P, P)],
                            kT_sb[:, dko, ts(mc, F)],
                            start=(dko == 0),
                            stop=(dko == DK_O - 1),
                        )
                    nc.vector.tensor_add(ps[:], ps[:], btile[:, ts(mc, F)])
                    nc.vector.reduce_max(
                        stats[:, mc : mc + 1], ps[:], axis=mybir.AxisListType.X
                    )
                nc.vector.reduce_max(
                    negmax[:], stats[:], axis=mybir.AxisListType.X, negate=True
                )

                # exp + per-chunk X-bar transpose of the probability tiles
                wT = mp.tile([P, M_BLOCKS, P], DT, tag="wT")  # [m_i, m_o, n]
                for mc in range(SC_CHUNKS):
                    nc.scalar.activation(
                        w16[:, ts(mc, F)],
                        sc_tiles[mc][:],
                        mybir.ActivationFunctionType.Exp,
                        bias=negmax[:, 0:1],
                        scale=1.0,
                        accum_out=sums[:, mc : mc + 1],
                    )
                    nc.sync.dma_start(
                        wT[:, 4 * mc : 4 * mc + 4, :],
                        w16[:, ts(mc, F)],
                        transpose=True,
                    )
                nc.vector.reduce_sum(rsum[:], sums[:], axis=mybir.AxisListType.X)
                nc.vector.reciprocal(rinv[:], rsum[:])

                # PV: out[n-block, dv] = wT.T @ v
                pv = psPV.tile([P, PV_CHUNKS, F], F32, tag="ps_pv")
                for c in range(PV_CHUNKS):
                    for mo in range(M_BLOCKS):
                        nc.tensor.matmul(
                            pv[:, c, :],
                            wT[:, mo, :],
                            v_sb[:, mo, ts(c, F)],
                            start=(mo == 0),
                            stop=(mo == M_BLOCKS - 1),
                        )
                outt = mp.tile([P, DV], F32, tag="outt")
                for c in range(PV_CHUNKS):
                    nc.vector.tensor_scalar_mul(
                        outt[:, ts(c, F)], pv[:, c, :], rinv[:, 0:1]
                    )
                nc.sync.dma_start(out_e[ds(blk * P, P), :], outt[:])

    nc.compile()
    return nc


_CACHE = {}


def _get_nc():
    if "nc" not in _CACHE:
        _CACHE["nc"] = build()
    return _CACHE["nc"]


def run(inputs, trace=False, trace_kwargs=None):
    nc = _get_nc()
    querys = np.ascontiguousarray(np.asarray(inputs["querys"], dtype=np.float32))
    keys = np.ascontiguousarray(np.asarray(inputs["keys"], dtype=np.float32))
    values = np.ascontiguousarray(np.asarray(inputs["values"], dtype=np.float32))
    mask = np.ascontiguousarray(np.asarray(inputs["mask"], dtype=np.int32))
    shared = {
        "mask": mask,
        "Wq": np.asarray(inputs["Wq"], dtype=np.float32),
        "Wk": np.asarray(inputs["Wk"], dtype=np.float32),
        "Wv": np.asarray(inputs["Wv"], dtype=np.float32),
        "bq": np.asarray(inputs["bq"], dtype=np.float32),
        "bk": np.asarray(inputs["bk"], dtype=np.float32),
    }
    in_maps = [
        {
            "querys": querys[b],
            "keys": keys[b],
            "values": values[b],
            **shared,
        }
        for b in range(B)
    ]
    res = run_bass_kernel_spmd(
        nc,
        in_maps,
        list(range(B)),
        trace=trace,
        **(trace_kwargs or {}),
    )
    out = np.stack([res.results[b]["out"] for b in range(B)]).astype(np.float32)
    # bv folded in on the host: softmax rows sum to 1, so W @ (v + bv) = W @ v + bv
    out += np.asarray(inputs["bv"], dtype=np.float32)[None, None, :]
    return out, res


def kernel(**inputs) -> np.ndarray:
    out, _ = run(inputs, trace=False)
    return out


if __name__ == "__main__":
    nc = _get_nc()
    print("built + compiled OK")
